# revision 12
# baseline (speedup 1.0000x reference)
"""BitNet ternary linear (nn_BitNetLinear4Bit) Trainium2 Bass kernel.

out = x @ (alpha * clip(round(w/alpha), -1, 1))^T + bias
  x: [2, 2048, 4096] f32, w: [11008, 4096] f32, alpha: [1] f32, bias: [11008] f32
  -> out: [2, 2048, 11008] f32

Sharding: column-parallel over 8 cores. Each core gets the full x
(replicated) and a 1376-row slice of w / bias; it produces a
[4096, 1376] slice of the output which the host concatenates.

Per-core algorithm (all math on device):
  Phase W: stream w-shard row-blocks, ternarize with two DVE ops
    (t = (w >= a/2) - (w <= -a/2), exact in bf16), PE-transpose the
    128x128 blocks, and keep the whole transposed ternary weight
    wT [128, 32, 1376] bf16 resident in SBUF.
  Phase MM: per 128-token block: DMA x rows (contiguous f32), cast to
    bf16, PE-transpose into xT [128, 32, 128]; accumulate K=4096 via 32
    bf16 matmuls into 3 PSUM tiles covering the 1376 output features;
    evict with one fused DVE op (psum * alpha + bias); DMA out.
"""

import numpy as np

B, S, DIN, DOUT = 2, 2048, 4096, 11008
NCORES = 8
DOUT_SH = DOUT // NCORES  # 1376
TOK = B * S  # 4096
P = 128
KO = DIN // P  # 32 contraction chunks
M_SUBS = TOK // P  # 32 token blocks
# output-feature tiles (psum free dim <= 512)
N_TILES = [(0, 512), (512, 512), (1024, 352)]
# w-shard row chunks of 128 (last is 96 rows, zero-padded)
W_CHUNKS = (DOUT_SH + P - 1) // P  # 11
# quantization column chunk
QCOL = 1024


def _build(TOK=TOK, DIN=DIN, DOUT_SH=DOUT_SH, debug=False):
    import concourse.mybir as mybir
    from concourse import bacc
    from concourse.tile import TileContext
    from concourse.masks import make_identity

    f32 = mybir.dt.float32
    bf16 = mybir.dt.bfloat16
    Alu = mybir.AluOpType

    KO = DIN // P
    M_SUBS = TOK // P
    N_TILES = []
    n0 = 0
    while n0 < DOUT_SH:
        N_TILES.append((n0, min(512, DOUT_SH - n0)))
        n0 += 512
    W_CHUNKS = (DOUT_SH + P - 1) // P
    QCOL = min(1024, DIN)

    nc = bacc.Bacc(None, target_bir_lowering=False, debug=debug)
    x_d = nc.dram_tensor("x", [TOK, DIN], f32, kind="ExternalInput")
    w_d = nc.dram_tensor("w", [DOUT_SH, DIN], f32, kind="ExternalInput")
    a_d = nc.dram_tensor("alpha", [1], f32, kind="ExternalInput")
    b_d = nc.dram_tensor("bias", [DOUT_SH], f32, kind="ExternalInput")
    o_d = nc.dram_tensor("out", [TOK, DOUT_SH], f32, kind="ExternalOutput")

    with TileContext(nc) as tc:
        with (
            tc.tile_pool(name="const", bufs=1) as const,
            tc.tile_pool(name="wres", bufs=1) as wres,
            tc.tile_pool(name="ptp", bufs=2, space="PSUM") as ptp,
        ):
            ident = const.tile([P, P], bf16)
            make_identity(nc, ident)
            alpha_sb = const.tile([P, 1], f32)
            nc.sync.dma_start(alpha_sb[:], a_d[:].to_broadcast((P, 1)))
            a2 = const.tile([P, 1], f32)
            nc.vector.tensor_scalar_mul(a2[:], alpha_sb[:], 0.5)
            na2 = const.tile([P, 1], f32)
            nc.vector.tensor_scalar_mul(na2[:], alpha_sb[:], -0.5)
            bias_sb = const.tile([P, DOUT_SH], f32)
            nc.sync.dma_start(
                bias_sb[:],
                b_d[:].rearrange("(a n) -> a n", a=1).to_broadcast((P, DOUT_SH)),
            )

            # resident transposed ternary weights: wT[p, ko, j] = t[j, ko*128+p]
            # split into two tensors along ko to keep per-tensor size <= 64KB/part
            KO_H = KO // 2
            wt_parts = [
                wres.tile([P, KO_H, DOUT_SH], bf16, name="wt_lo"),
                wres.tile([P, KO - KO_H, DOUT_SH], bf16, name="wt_hi"),
            ]

            def wt_slice(ko):
                return wt_parts[ko // KO_H][:, ko % KO_H, :]

            # ---- Phase W: quantize + transpose w shard ----
            with tc.tile_pool(name="wq", bufs=2) as wq:
                for c in range(W_CHUNKS):
                    rc = min(P, DOUT_SH - c * P)  # 128 or 96 (last)
                    for q in range(DIN // QCOL):
                        wrow = wq.tile([P, QCOL], f32, tag="wrow")
                        if rc < P:
                            nc.any.memset(wrow[:], 0.0)
                        nc.sync.dma_start(
                            wrow[:rc, :],
                            w_d[c * P : c * P + rc, q * QCOL : (q + 1) * QCOL],
                        )
                        le = wq.tile([P, QCOL], bf16, tag="le")
                        nc.vector.tensor_tensor(
                            le[:], wrow[:], na2[:, 0:1].to_broadcast((P, QCOL)), Alu.is_le
                        )
                        ge = wq.tile([P, QCOL], bf16, tag="ge")
                        nc.vector.tensor_tensor(
                            ge[:], wrow[:], a2[:, 0:1].to_broadcast((P, QCOL)), Alu.is_ge
                        )
                        tq = wq.tile([P, QCOL], bf16, tag="tq")
                        nc.vector.tensor_sub(tq[:], ge[:], le[:])
                        for bb in range(QCOL // P):
                            ko = q * (QCOL // P) + bb
                            pt = ptp.tile([P, P], bf16, tag="pt")
                            nc.tensor.transpose(
                                pt[:], tq[:, bb * P : (bb + 1) * P], ident[:]
                            )
                            nc.any.tensor_copy(
                                wt_slice(ko)[:, c * P : c * P + rc], pt[:, :rc]
                            )

            # ---- Phase MM ----
            with (
                tc.tile_pool(name="xp", bufs=2) as xp,
                tc.tile_pool(name="op", bufs=2) as op,
                tc.tile_pool(name="pso", bufs=6, space="PSUM") as pso,
            ):
                for ms in range(M_SUBS):
                    xbf = xp.tile([P, DIN], bf16, tag="xbf")
                    for h in range(2):
                        hw = DIN // 2
                        xrow = xp.tile([P, hw], f32, tag="xrow")
                        nc.sync.dma_start(
                            xrow[:], x_d[ms * P : (ms + 1) * P, h * hw : (h + 1) * hw]
                        )
                        nc.any.tensor_copy(xbf[:, h * hw : (h + 1) * hw], xrow[:])
                    xt = xp.tile([P, KO, P], bf16, tag="xt")
                    for ko in range(KO):
                        pt2 = ptp.tile([P, P], bf16, tag="pt")
                        nc.tensor.transpose(
                            pt2[:], xbf[:, ko * P : (ko + 1) * P], ident[:]
                        )
                        nc.any.tensor_copy(xt[:, ko, :], pt2[:])

                    psums = []
                    for i, (n0, nsz) in enumerate(N_TILES):
                        po = pso.tile([P, 512], f32, tag="po", name=f"po_{ms}_{i}")
                        psums.append(po)
                    for ko in range(KO):
                        for i, (n0, nsz) in enumerate(N_TILES):
                            nc.tensor.matmul(
                                psums[i][:, :nsz],
                                xt[:, ko, :],
                                wt_slice(ko)[:, n0 : n0 + nsz],
                                start=(ko == 0),
                                stop=(ko == KO - 1),
                            )
                    out_sb = op.tile([P, DOUT_SH], f32, tag="osb")
                    for i, (n0, nsz) in enumerate(N_TILES):
                        nc.vector.tensor_tensor(
                            out_sb[:, n0 : n0 + nsz],
                            psums[i][:, :nsz],
                            alpha_sb[:, 0:1].to_broadcast((P, nsz)),
                            Alu.mult,
                        )
                        nc.vector.tensor_add(
                            out_sb[:, n0 : n0 + nsz],
                            out_sb[:, n0 : n0 + nsz],
                            bias_sb[:, n0 : n0 + nsz],
                        )
                    nc.sync.dma_start(o_d[ms * P : (ms + 1) * P, :], out_sb[:])

    nc.compile()
    return nc


_CACHE = {}


def _get_nc():
    if "nc" not in _CACHE:
        _CACHE["nc"] = _build()
    return _CACHE["nc"]


def kernel(x, w, alpha, bias):
    from concourse.bass_utils import run_bass_kernel_spmd

    nc = _get_nc()
    x2 = np.ascontiguousarray(np.asarray(x, dtype=np.float32).reshape(TOK, DIN))
    alpha2 = np.ascontiguousarray(np.asarray(alpha, dtype=np.float32).reshape(1))
    in_maps = []
    for c in range(NCORES):
        in_maps.append(
            {
                "x": x2,
                "w": np.ascontiguousarray(w[c * DOUT_SH : (c + 1) * DOUT_SH]),
                "alpha": alpha2,
                "bias": np.ascontiguousarray(bias[c * DOUT_SH : (c + 1) * DOUT_SH]),
            }
        )
    res = run_bass_kernel_spmd(nc, in_maps, core_ids=list(range(NCORES)))
    outs = [res.results[c]["out"] for c in range(NCORES)]
    out = np.concatenate(outs, axis=1).reshape(B, S, DOUT)
    return np.ascontiguousarray(out.astype(np.float32))
